# revision 11
# baseline (speedup 1.0000x reference)
"""DeepTensorNeuralNetwork (DTNN / gnn_message_passing) Trainium2 kernel.

Math (per reference):
    d_sum = distance.sum(axis=2)                                  # (B,N,R)
    for l in 0..2:
        cf = x @ Wcf[l].T + bcf[l]                                # (B,N,H)
        df = d_sum @ Wdf[l].T + N*bdf[l]                          # (B,N,H)
        h  = (cf*df) @ Wfc[l].T                                   # (B,N,F)
        x  = h + tanh(h)
    g = x.sum(axis=1); out = (g @ fc0.T + b0) @ ow.T + ob         # (B,1)

Strategy: data-parallel over batch across 8 NeuronCores (8 batches each).
`distance` is cast to fp16 on the host (the accuracy gate is ~2e-2; the
quantization error of the j-sum is ~1e-4 relative), which halves the
dominant HBM stream to 2.1 MB/batch (~4.8us at the ~433 GB/s measured
per-core rate).  The j-reduction is a DVE binary fold tree, all levels in
the fp16 2x mode (~0.52 ns/output-lane), stopping at 128 cols = (j2, r):
the final 2:1 j-fold is absorbed into the df matmul by doubling the Wdf
lhsT rows (contract 128 instead of 64).  Optionally GpSimd absorbs the
level-1 fold of a few early batches (GP_SET) to keep DVE at the DMA pace.

The 3-layer pipeline runs per batch-group (4,2,2).  Residual adds never
run on DVE: x_{l+1} = h + tanh(h) is consumed only by matmuls, so the
next layer's cf accumulates mm(Wcf,h_sb) + mm(Wcf,th) in PSUM (linearity)
with ACT producing h_sb (Copy) and th (Tanh) from h's PSUM.  Groups 0/1
use ACT bias-copies so the DVE cf*df muls read fp16 SBUF in 2x mode; the
final group instead folds biases into the matmuls (ones-row rhs) and its
muls read fp32 PSUM directly, removing two ACT hops per layer from the
post-stream critical chain.  PSUM: two bank sets A/B (cf,df,h each);
G0->A, G1->B (c-halves share a bank serially), the final group uses A+B
in parallel; tr + hd shared = 8 banks.  Emission is interleaved so each
in-order engine queue reaches ops with deps nearly met (no fold op is
queued behind layer work that isn't ready, and vice versa).
"""

import numpy as np

B, N, F, R, H = 64, 128, 128, 64, 256
L = 3
NCORES = 8
BL = B // NCORES   # batches per core
GROUPS = ((0, 1, 2, 3), (4, 5), (6, 7))
GP_SET = ()        # GpSimd fold assist: measured ~3.8 ns/elem (10x slower
                   # than DVE 2x) -- gating any fold level on it loses time

# wpack layout, fp32 columns (bf-cols = fp16-element columns of the
# bitcast view at 2x the fp32 column index):
#   [0, 384)      wcf lhsT f16 : bf-col l*H+h           = Wcf_w[l, h, f]
#   [384, 768)    wfc lhsT f16 : bf-col (l*2+c)*F+f     = Wfc_w[l, f, c*128+hc]
#   [768, 774)    cf bias fp32 : col l*2+c              = Wcf_b[l, c*128+h]
#   [774, 780)    df bias fp32 : col l*2+c              = N * Wdf_b[l, c*128+h]
#   [780, 781)    head lhsT f16: bf-col 0               = (out_w @ fc0_w)[0, f]
#   [784, 1168)   wdf2 lhsT f16: bf-col l*H+h, row j2*64+r = Wdf_w[l, h, r]
#   [1168, 1680)  x f16        : bf-col b*N+n           = x[b_local, n, f]
#   [1680, 1744)  identity f16-packed
#   [1744, 2000)  bias rows f16: idx k (cf: k=l*2+c, df: k=6+l*2+c) at
#                 row 32*(k%3), bf-cols [128*(k//3), 128*(k//3)+128)
BCF_OFF = 768
BDF_OFF = 774
HEAD_OFF = 780
WDF_OFF = 784
XOFF = 1168
IDOFF = 1680
BROW_OFF = 1744
WCOLS = 2000

_CACHE = {}


def _build_program():
    import concourse.bass as bass
    from concourse import bacc
    import concourse.tile as tile
    from concourse import mybir

    f32 = mybir.dt.float32
    f16 = mybir.dt.float16
    AX = mybir.AxisListType
    AF = mybir.ActivationFunctionType

    nc = bacc.Bacc("TRN2")
    dist = nc.declare_dram_parameter("dist", [BL, N, N * R], f16, isOutput=False)
    wpack = nc.declare_dram_parameter("wpack", [128, WCOLS], f32, isOutput=False)
    out_ext = nc.declare_dram_parameter("out", [BL, 1], f32, isOutput=True)

    with tile.TileContext(nc) as tc:
        with (
            tc.tile_pool(name="consts", bufs=1) as consts,
            tc.tile_pool(name="dist", bufs=4) as dist_pool,
            tc.tile_pool(name="fold", bufs=2) as fold_pool,
            tc.tile_pool(name="dsum", bufs=2) as dsum_pool,
            tc.tile_pool(name="work", bufs=2) as work,
            tc.tile_pool(name="psA", bufs=1, space="PSUM") as psA,
            tc.tile_pool(name="psB", bufs=1, space="PSUM") as psB,
            tc.tile_pool(name="psS", bufs=1, space="PSUM") as psS,
        ):
            # ---- DMA: distance stream on sync+scalar rings ------------
            dist_tiles = {}

            def start_dist_dma(b):
                t = dist_pool.tile([N, N * R], f16, tag="dist", name="dist_t")
                dflat = dist[b, :, :]
                nchunk = 2
                cw = (N * R) // nchunk
                for k in range(nchunk):
                    eng = nc.sync if (k % 2 == 0) else nc.scalar
                    eng.dma_start(out=t[:, k * cw : (k + 1) * cw],
                                  in_=dflat[:, k * cw : (k + 1) * cw])
                dist_tiles[b] = t

            start_dist_dma(0)
            start_dist_dma(1)

            wp = consts.tile([128, WCOLS], f32)
            hwc = WCOLS // 2
            nc.sync.dma_start(out=wp[:, 0:hwc], in_=wpack[:, 0:hwc])
            nc.scalar.dma_start(out=wp[:, hwc:WCOLS], in_=wpack[:, hwc:WCOLS])
            wb = wp.bitcast(f16)  # (128, 2*WCOLS) f16 view
            ident = wb[:, 2 * IDOFF : 2 * IDOFF + 128]
            out_acc = consts.tile([1, BL], f32)

            start_dist_dma(2)

            def wcf_l(l, c):
                o = l * H + c * 128
                return wb[:, o : o + 128]

            def wdf_l(l, c):
                o = 2 * WDF_OFF + l * H + c * 128
                return wb[:, o : o + 128]

            def wfc_l(l, c):
                o = 2 * 384 + (l * 2 + c) * F
                return wb[:, o : o + F]

            def bcf_l(l, c):
                o = BCF_OFF + l * 2 + c
                return wp[:, o : o + 1]

            def bdf_l(l, c):
                o = BDF_OFF + l * 2 + c
                return wp[:, o : o + 1]

            def _brow(k):
                r = 32 * (k % 3)
                co = 2 * BROW_OFF + 128 * (k // 3)
                return wb[r : r + 1, co : co + 128], r

            def bcf_row(l, c):
                return _brow(l * 2 + c)

            def bdf_row(l, c):
                return _brow(6 + l * 2 + c)

            head_w = wb[:, 2 * HEAD_OFF : 2 * HEAD_OFF + 1]

            def xcols(b0, b1):
                return wb[:, 2 * XOFF + b0 * N : 2 * XOFF + b1 * N]

            # ---- folds -------------------------------------------------
            dsums = {}

            def fold_full(b, gp=False):
                """One tree 8192 -> 128 cols (j2, r); opt. level1 on GpSimd."""
                src = dist_tiles.pop(b)
                if b + 3 < BL:
                    start_dist_dma(b + 3)
                s = fold_pool.tile([N, 4096], f16, tag="s", name="s")
                if gp:
                    nc.gpsimd.tensor_add(s[:, 0:2048], src[:, 0:2048],
                                         src[:, 4096:6144])
                    nc.vector.tensor_add(s[:, 2048:4096], src[:, 2048:4096],
                                         src[:, 6144:8192])
                else:
                    nc.vector.tensor_add(s, src[:, 0:4096], src[:, 4096:8192])
                t = fold_pool.tile([N, 2048], f16, tag="t", name="t")
                dsum = dsum_pool.tile([N, 128], f16, tag="dsum", name="dsum", bufs=6)
                cur, other, w = s, t, 2048
                while w >= 128:
                    dst = dsum if w == 128 else other[:, 0:w]
                    nc.vector.tensor_add(dst, cur[:, 0:w], cur[:, w : 2 * w])
                    cur, other = other, cur
                    w //= 2
                dsums[b] = dsum

            def fold_halves(b):
                """Two half-trees (j<64 | j>=64) -> dsum halves; tracks the
                2-chunk DMA so the last-arriving half gates only itself."""
                src_t = dist_tiles.pop(b)
                if b + 3 < BL:
                    start_dist_dma(b + 3)
                dsum = dsum_pool.tile([N, 128], f16, tag="dsum",
                                      name="dsumh", bufs=6)
                for hf in range(2):
                    off = hf * 4096
                    s = fold_pool.tile([N, 2048], f16, tag=f"hs{hf}", name="hs")
                    nc.vector.tensor_add(s, src_t[:, off : off + 2048],
                                         src_t[:, off + 2048 : off + 4096])
                    t = fold_pool.tile([N, 1024], f16, tag=f"ht{hf}", name="ht")
                    cur, other, w = s, t, 1024
                    while w >= 64:
                        dst = dsum[:, hf * 64 : hf * 64 + 64] if w == 64 \
                            else other[:, 0:w]
                        nc.vector.tensor_add(dst, cur[:, 0:w], cur[:, w : 2 * w])
                        cur, other = other, cur
                        w //= 2
                dsums[b] = dsum

            # ---- group state / layer pipeline --------------------------
            gstate = {}

            def ps_pool(gi, c=0):
                """G0 -> A, G1 -> B, final group -> A (c=0) / B (c=1)."""
                if gi == 0:
                    return psA
                if gi == 1:
                    return psB
                return psA if c == 0 else psB

            def emit_trs(gi):
                bs = GROUPS[gi]
                NG = len(bs) * N
                dsT = dsum_pool.tile([128, 4 * N], f16, tag="dsT",
                                     name=f"dsT{gi}")
                for k, b in enumerate(bs):
                    trp = psS.tile([128, N], f16, tag="tr", name="trp")
                    nc.tensor.transpose(trp, dsums.pop(b), ident)
                    nc.scalar.activation(
                        out=dsT[:, k * N : (k + 1) * N], in_=trp, func=AF.Copy
                    )
                gstate[gi] = {"dsT": dsT[:, 0:NG], "NG": NG, "bs": bs,
                              "xc": xcols(bs[0], bs[-1] + 1)}

            def emit_layer(gi, l):
                """Throughput path (G0/G1): ACT bias copies, f16 SBUF muls.
                c halves share one cf bank and one df bank serially."""
                st = gstate[gi]
                NG = st["NG"]
                ms = []
                for c in range(2):
                    cfp = ps_pool(gi, 0).tile([128, 4 * N], f32, tag="cf",
                                              name="cfp")[:, 0:NG]
                    if l == 0:
                        nc.tensor.matmul(cfp, wcf_l(l, c), st["xc"],
                                         start=True, stop=True)
                    else:
                        nc.tensor.matmul(cfp, wcf_l(l, c), st["hsb"],
                                         start=True, stop=False)
                        nc.tensor.matmul(cfp, wcf_l(l, c), st["th"],
                                         start=False, stop=True)
                    cfs = work.tile([128, 4 * N], f16, tag=f"cfs{gi % 2}{c}",
                                    name="cfs")[:, 0:NG]
                    nc.scalar.activation(out=cfs, in_=cfp, func=AF.Identity,
                                         bias=bcf_l(l, c))
                    dfp = ps_pool(gi, 1).tile([128, 4 * N], f32, tag="df",
                                              name="dfp")[:, 0:NG]
                    nc.tensor.matmul(dfp, wdf_l(l, c), st["dsT"],
                                     start=True, stop=True)
                    dfs = work.tile([128, 4 * N], f16, tag=f"dfs{gi % 2}{c}",
                                    name="dfs")[:, 0:NG]
                    nc.scalar.activation(out=dfs, in_=dfp, func=AF.Identity,
                                         bias=bdf_l(l, c))
                    m = work.tile([128, 4 * N], f16, tag=f"m{gi % 2}{c}",
                                  name="m")[:, 0:NG]
                    meng = nc.gpsimd if gi == 0 else nc.vector
                    meng.tensor_mul(m, cfs, dfs)
                    ms.append(m)
                _emit_h(gi, l, ms)

            def _emit_h(gi, l, ms):
                st = gstate[gi]
                NG = st["NG"]
                hpool = psA if gi == 0 else psB
                hp = hpool.tile([F, 4 * N], f32, tag="h", name="hp")[:, 0:NG]
                nc.tensor.matmul(hp, wfc_l(l, 0), ms[0], start=True, stop=False)
                nc.tensor.matmul(hp, wfc_l(l, 1), ms[1], start=False, stop=True)
                hsb = work.tile([F, 4 * N], f16, tag=f"hsb{gi % 2}",
                                name="hsb")[:, 0:NG]
                nc.scalar.activation(out=hsb, in_=hp, func=AF.Copy)
                th = work.tile([F, 4 * N], f16, tag=f"th{gi % 2}",
                               name="th")[:, 0:NG]
                nc.scalar.activation(out=th, in_=hp, func=AF.Tanh)
                st["hsb"], st["th"] = hsb, th

            def emit_head(gi):
                st = gstate[gi]
                NG, bs = st["NG"], st["bs"]
                G = len(bs)
                hd = psS.tile([1, 4 * N], f32, tag="hd", name="hd")[:, 0:NG]
                nc.tensor.matmul(hd, head_w, st["hsb"], start=True, stop=False)
                nc.tensor.matmul(hd, head_w, st["th"], start=False, stop=True)
                nc.vector.tensor_reduce(
                    out=out_acc[0:1, bs[0] : bs[0] + G],
                    in_=hd.rearrange("o (b n) -> o b n", b=G),
                    axis=AX.X,
                    op=mybir.AluOpType.add,
                )

            # ---- tail-group latency path (single-batch groups) ---------
            # The critical input per layer is cf (depends on previous layer's
            # h), so cf accumulates in PSUM via a bias-matmul (ones-row rhs)
            # + mm(hsb) + mm(th); df is dsT-only and is precomputed into
            # SBUF f16 (ACT bias copy) off the critical path.  The DVE mul
            # reads dfs(SBUF) x cfp(PSUM) -- one PSUM operand, 1x mode.
            gl = {}

            def emit_glast_pre(gi):
                """x-only work: layer-0 cf into SBUF f16."""
                bs = GROUPS[gi]
                NG = len(bs) * N
                st = gl.setdefault(gi, {})
                st["NG"], st["bs"] = NG, bs
                cfs0 = []
                for c in range(2):
                    cfp = ps_pool(gi, c).tile([128, 4 * N], f32, tag="cf",
                                              name="cfpL")[:, 0:NG]
                    nc.tensor.matmul(cfp, wcf_l(0, c), xcols(bs[0], bs[-1] + 1),
                                     start=True, stop=True)
                    cs = work.tile([128, 2 * N], f16, tag=f"glcf{c}",
                                   name="glcfs", bufs=1)[:, 0:NG]
                    nc.scalar.activation(out=cs, in_=cfp, func=AF.Identity,
                                         bias=bcf_l(0, c))
                    cfs0.append(cs)
                st["cfs0"] = cfs0

            def emit_glast_trs(gi):
                st = gl[gi]
                bs, NG = st["bs"], st["NG"]
                dsT = dsum_pool.tile([128, 4 * N], f16, tag="dsT",
                                     name=f"dsTL{gi}")
                for k, b in enumerate(bs):
                    trp = psS.tile([128, N], f16, tag="tr", name="trpL")
                    nc.tensor.transpose(trp, dsums.pop(b), ident)
                    nc.scalar.activation(
                        out=dsT[:, k * N : (k + 1) * N], in_=trp, func=AF.Copy
                    )
                st["dsT"] = dsT[:, 0:NG]

            def emit_glast_dfs(gi, l):
                """Precompute df for layer l>=1 into SBUF (off-critical)."""
                st = gl[gi]
                NG = st["NG"]
                res = []
                for c in range(2):
                    dfp = ps_pool(gi, c).tile([128, 4 * N], f32, tag="df",
                                              name="dfpL")[:, 0:NG]
                    nc.tensor.matmul(dfp, wdf_l(l, c), st["dsT"],
                                     start=True, stop=True)
                    ds = work.tile([128, 2 * N], f16, tag=f"gldf{c}{l}",
                                   name="gldfs", bufs=1)[:, 0:NG]
                    nc.scalar.activation(out=ds, in_=dfp, func=AF.Identity,
                                         bias=bdf_l(l, c))
                    res.append(ds)
                st[f"dfs{l}"] = res

            def emit_glast_layer(gi, l):
                st = gl[gi]
                NG = st["NG"]
                ms = []
                for c in range(2):
                    m = work.tile([128, 2 * N], f16, tag=f"glm{c}",
                                  name="glm", bufs=2)[:, 0:NG]
                    if l == 0:
                        # df in PSUM; bias + mul fused on DVE; cf from SBUF
                        dfp = ps_pool(gi, c).tile([128, 4 * N], f32, tag="df",
                                                  name="dfpL")[:, 0:NG]
                        nc.tensor.matmul(dfp, wdf_l(l, c), st["dsT"],
                                         start=True, stop=True)
                        nc.vector.scalar_tensor_tensor(
                            out=m, in0=dfp, scalar=bdf_l(l, c),
                            in1=st["cfs0"][c],
                            op0=mybir.AluOpType.add, op1=mybir.AluOpType.mult)
                    else:
                        # cf in PSUM; bias + mul fused; df from SBUF
                        nc.vector.scalar_tensor_tensor(
                            out=m, in0=st["cfp"][c], scalar=bcf_l(l, c),
                            in1=st[f"dfs{l}"][c],
                            op0=mybir.AluOpType.add, op1=mybir.AluOpType.mult)
                    ms.append(m)
                hp = (psA if l % 2 == 0 else psB).tile(
                    [F, 4 * N], f32, tag="h", name="hpL")[:, 0:NG]
                nc.tensor.matmul(hp, wfc_l(l, 0), ms[0], start=True, stop=False)
                nc.tensor.matmul(hp, wfc_l(l, 1), ms[1], start=False, stop=True)
                hsb = work.tile([F, 2 * N], f16, tag="glhsb", name="glhsb",
                                bufs=2)[:, 0:NG]
                nc.scalar.activation(out=hsb, in_=hp, func=AF.Copy)
                th = work.tile([F, 2 * N], f16, tag="glth", name="glth",
                               bufs=2)[:, 0:NG]
                nc.scalar.activation(out=th, in_=hp, func=AF.Tanh)
                if l < L - 1:
                    cfps = []
                    for c in range(2):
                        cfp = ps_pool(gi, c).tile([128, 4 * N], f32, tag="cf",
                                                  name="cfpL")[:, 0:NG]
                        nc.tensor.matmul(cfp, wcf_l(l + 1, c), hsb,
                                         start=True, stop=False)
                        nc.tensor.matmul(cfp, wcf_l(l + 1, c), th,
                                         start=False, stop=True)
                        cfps.append(cfp)
                    st["cfp"] = cfps
                else:
                    st["hsb"], st["th"] = hsb, th

            def emit_glast_head(gi):
                st = gl[gi]
                NG, bs = st["NG"], st["bs"]
                G = len(bs)
                hd = psS.tile([1, 4 * N], f32, tag="hd", name="hdL")[:, 0:NG]
                nc.tensor.matmul(hd, head_w, st["hsb"], start=True, stop=False)
                nc.tensor.matmul(hd, head_w, st["th"], start=False, stop=True)
                nc.vector.tensor_reduce(
                    out=out_acc[0:1, bs[0] : bs[0] + G],
                    in_=hd.rearrange("o (b n) -> o b n", b=G),
                    axis=AX.X,
                    op=mybir.AluOpType.add,
                )

            # ---- schedule ---------------------------------------------
            fold_full(0)
            fold_full(1, gp=1 in GP_SET)
            fold_full(2, gp=2 in GP_SET)
            emit_glast_pre(2)
            fold_full(3, gp=3 in GP_SET)
            emit_trs(0)
            emit_layer(0, 0)
            fold_full(4, gp=4 in GP_SET)
            emit_layer(0, 1)
            fold_full(5, gp=5 in GP_SET)
            emit_layer(0, 2)
            emit_head(0)
            emit_trs(1)
            emit_layer(1, 0)
            fold_full(6, gp=6 in GP_SET)
            emit_layer(1, 1)
            fold_halves(BL - 1)
            emit_layer(1, 2)
            emit_head(1)
            emit_glast_trs(2)
            emit_glast_layer(2, 0)
            emit_glast_dfs(2, 1)
            emit_glast_layer(2, 1)
            emit_glast_dfs(2, 2)
            emit_glast_layer(2, 2)
            emit_glast_head(2)

            nc.sync.dma_start(out=out_ext.rearrange("b o -> o b"), in_=out_acc)

    return nc


def _host_pack(x, Wcf_w, Wcf_b, Wdf_w, Wdf_b, Wfc_w, fc0_w, fc0_b, out_w, out_b):
    f = np.float32
    h = np.float16

    def pack_bf(a):  # (rows, 2K) f16 -> (rows, K) fp32 bit-packed
        return np.ascontiguousarray(a.astype(h)).view(f)

    base = np.zeros((128, WCOLS), f)
    base[:, 0:384] = pack_bf(np.asarray(Wcf_w, f).transpose(2, 0, 1).reshape(128, L * H))
    base[:, 384:768] = pack_bf(
        np.asarray(Wfc_w, f).reshape(L, F, 2, 128).transpose(3, 0, 2, 1).reshape(128, L * 2 * F)
    )
    base[:, BCF_OFF : BCF_OFF + 6] = (
        np.asarray(Wcf_b, f).reshape(L, 2, 128).transpose(2, 0, 1).reshape(128, 6)
    )
    base[:, BDF_OFF : BDF_OFF + 6] = (
        (N * np.asarray(Wdf_b, f)).reshape(L, 2, 128).transpose(2, 0, 1).reshape(128, 6)
    )
    w_head = (np.asarray(out_w, np.float64) @ np.asarray(fc0_w, np.float64))[0]  # (F,)
    head_pair = np.zeros((128, 2), f)
    head_pair[:, 0] = w_head.astype(f)
    base[:, HEAD_OFF : HEAD_OFF + 1] = pack_bf(head_pair)
    # wdf2: rows (j2*64 + r) both halves = Wdf_w[l, h, r]
    wdf2 = np.zeros((128, L * H), f)
    wt = np.asarray(Wdf_w, f).transpose(2, 0, 1).reshape(R, L * H)
    wdf2[0:R] = wt
    wdf2[R:128] = wt
    base[:, WDF_OFF : WDF_OFF + 384] = pack_bf(wdf2)
    base[:, IDOFF : IDOFF + 64] = pack_bf(np.eye(128, dtype=f))
    # bias rows for the final group's bias-matmuls: idx k at row 32*(k%3),
    # f16 cols [128*(k//3), +128)
    bvals = np.concatenate([
        np.asarray(Wcf_b, f).reshape(6, 128),
        (N * np.asarray(Wdf_b, f)).reshape(6, 128),
    ])  # (12, 128)
    brow = np.zeros((128, 512), f)
    for k in range(12):
        brow[32 * (k % 3), 128 * (k // 3) : 128 * (k // 3) + 128] = bvals[k]
    base[:, BROW_OFF : BROW_OFF + 256] = pack_bf(brow)

    b_head = float((np.asarray(out_w, np.float64) @ np.asarray(fc0_b, np.float64)
                    + np.asarray(out_b, np.float64)).reshape(()))

    x_t = np.asarray(x, f).transpose(0, 2, 1)  # (B, F, N)
    wpacks = []
    for i in range(NCORES):
        wp = base.copy()
        wp[:, XOFF : XOFF + BL * N // 2] = pack_bf(
            x_t[i * BL : (i + 1) * BL].transpose(1, 0, 2).reshape(128, BL * N)
        )
        wpacks.append(wp)
    return wpacks, b_head


def run(trace=False, **inputs):
    from concourse.bass_utils import run_bass_kernel_spmd

    dist16 = np.ascontiguousarray(
        np.asarray(inputs["distance"]).astype(np.float16).reshape(B, N, N * R)
    )
    wpacks, b_head = _host_pack(
        inputs["x"], inputs["Wcf_w"], inputs["Wcf_b"], inputs["Wdf_w"], inputs["Wdf_b"],
        inputs["Wfc_w"], inputs["fc0_w"], inputs["fc0_b"], inputs["out_w"], inputs["out_b"],
    )

    if "nc" not in _CACHE:
        nc = _build_program()
        nc.finalize()
        _CACHE["nc"] = nc
    nc = _CACHE["nc"]

    in_maps = []
    for i in range(NCORES):
        in_maps.append({
            "dist": np.ascontiguousarray(dist16[i * BL : (i + 1) * BL]),
            "wpack": wpacks[i],
        })
    res = run_bass_kernel_spmd(nc, in_maps, list(range(NCORES)), trace=trace)
    out = np.concatenate([res.results[i]["out"] for i in range(NCORES)], axis=0)
    out = (out.astype(np.float64) + b_head).astype(np.float32)
    return out, res


def kernel(**inputs):
    out, _ = run(trace=False, **inputs)
    return out
